# revision 5
# baseline (speedup 1.0000x reference)
"""Distributed causal multi-head attention block on 8 TRN2 NeuronCores.

Tensor-parallel over heads (2 heads/core, core r holds global heads
2r, 2r+1).  Schedule per core:
  - QKV row-block pairs interleaved with head-0 attention windows, so
    ScalarE exp starts ~20us into the run instead of after all of QKV:
    rb0 rb1 W(b0,q0) rb2 rb3 W(b0,q1) rb4 rb5 W(b1,q0) rb6 rb7 W(b1,q1).
  - Attention windows are software-pipelined: scores(c+1) is emitted
    before AV(c) so the PE never stalls behind exp(c) on ScalarE.
    Scores in [k,q] layout, exp on ScalarE (1/8 folded), diagonal
    triangle masked on DVE, AV accumulated over key chunks with a ones
    column in v giving softmax row sums (Z) for free.
  - Raw [65, q] (att|Z) tiles ship UNNORMALIZED through a per-head
    AllToAll; h0's A2A overlaps h1's attention, h1's A2A overlaps h0's
    projection half.  Post-A2A, Z rows are gathered compactly [32, 128],
    reciprocal'd in one DVE op, broadcast via a DRAM-bounce DMA, and
    applied as per-kc bf16 multiplies; the output projection accumulates
    h0's cin chunks then h1's (kc-outer so h1 chunks start as soon as
    their normalize lands), with the t-shard DMA'd out per 128-row block.
"""

import numpy as np
import ml_dtypes

import concourse.bass as bass
import concourse.mybir as mybir
import concourse.tile as tile
from concourse import bacc
from concourse.bass_utils import run_bass_kernel_spmd

P = 128
B, T, C = 2, 2048, 1024
H, D = 16, 64
NCORES = 8
HPC = 2                    # heads per core
BT = B * T                 # 4096
KC = C // P                # 8 contraction chunks
NRB = BT // 512            # 8 row blocks of 512
QW = 1024                  # query window
TSH = 512                  # output t-shard rows per core
F32 = mybir.dt.float32
BF16 = mybir.dt.bfloat16
SCALE = 1.0 / 8.0


def build_nc():
    nc = bacc.Bacc(None, target_bir_lowering=False)

    xT = nc.dram_tensor("xT", [C, BT], BF16, kind="ExternalInput")
    w_qk = nc.dram_tensor("w_qk", [C, 2 * P], BF16, kind="ExternalInput")
    w_v = nc.dram_tensor("w_v", [C, P], BF16, kind="ExternalInput")
    b_qk = nc.dram_tensor("b_qk", [2 * P], F32, kind="ExternalInput")
    b_v = nc.dram_tensor("b_v", [P], F32, kind="ExternalInput")
    w_pr = nc.dram_tensor("w_proj", [C, C], BF16, kind="ExternalInput")
    b_pr = nc.dram_tensor("b_proj", [C], F32, kind="ExternalInput")
    maskm = nc.dram_tensor("mask", [P, P], BF16, kind="ExternalInput")
    out = nc.dram_tensor("out", [TSH, C], F32, kind="ExternalOutput")

    groups = [list(range(NCORES))]

    with tile.TileContext(nc) as tc:
        with (
            tc.tile_pool(name="consts", bufs=1) as consts,
            tc.tile_pool(name="persist", bufs=1) as persist,
            tc.tile_pool(name="xtg", bufs=3) as xtg_pool,
            tc.tile_pool(name="pt", bufs=3) as pt_pool,
            tc.tile_pool(name="stage", bufs=3) as stage_pool,
            tc.tile_pool(name="small", bufs=2) as small_pool,
            tc.tile_pool(name="ps_a", bufs=2, space="PSUM") as ps_a,
            tc.tile_pool(name="ps_b", bufs=2, space="PSUM") as ps_b,
            tc.tile_pool(name="dram", bufs=1, space="DRAM") as dram,
        ):
            # ---- constants (W_proj + b_proj deferred until after A2A#1) ----
            wqk_sb = consts.tile([P, KC, 2 * P], BF16)
            nc.sync.dma_start(wqk_sb[:], w_qk.ap().rearrange("(kc p) m -> p kc m", p=P))
            wv_sb = consts.tile([P, KC, P], BF16)
            nc.sync.dma_start(wv_sb[:], w_v.ap().rearrange("(kc p) m -> p kc m", p=P))
            bqk_sb = consts.tile([P, 2], F32)
            nc.sync.dma_start(bqk_sb[:], b_qk.ap().rearrange("(m p) -> p m", p=P))
            bv_sb = consts.tile([P, P], F32)
            nc.sync.dma_start(
                bv_sb[:],
                b_v.ap().rearrange("(o m) -> o m", o=1).to_broadcast((P, P)),
            )
            mask_sb = consts.tile([P, P], BF16)
            nc.sync.dma_start(mask_sb[:], maskm.ap())

            # persistent phase-1 outputs
            # qkT/kT: [p=(h,d), t];  vext: [p=t%128, chunk, head, 65]
            qkT = persist.tile([P, BT], BF16)
            kT = persist.tile([P, BT], BF16)
            vext = persist.tile([P, BT // P, HPC, 65], BF16)
            nc.vector.memset(vext[:, :, :, 64], 1.0)

            xT_blocked = xT.ap().rearrange(
                "(kh kc p) (r t) -> r kh p kc t", p=P, r=NRB, kh=2
            )

            a2a_in = [
                dram.tile([NCORES * 65, TSH], BF16, name=f"a2a_in_{h}")
                for h in range(HPC)
            ]
            a2a_out = [
                dram.tile([NCORES * 65, TSH], BF16, name=f"a2a_out_{h}")
                for h in range(HPC)
            ]

            def emit_rb(rb):
                xtg = xtg_pool.tile([P, KC, 512], BF16, name="xtg")
                nc.sync.dma_start(xtg[:, 0:KC // 2, :], xT_blocked[rb, 0])
                nc.sync.dma_start(xtg[:, KC // 2:KC, :], xT_blocked[rb, 1])
                ps_qk = ps_a.tile([P, QW], F32, name="ps_qk", tag="a")
                for kc in range(KC):
                    for m in range(2):
                        nc.tensor.matmul(
                            ps_qk[:, m * 512:(m + 1) * 512],
                            lhsT=wqk_sb[:, kc, m * P:(m + 1) * P],
                            rhs=xtg[:, kc, :],
                            start=(kc == 0),
                            stop=(kc == KC - 1),
                        )
                for m, dst in ((0, qkT), (1, kT)):
                    nc.vector.tensor_scalar(
                        dst[:, rb * 512:(rb + 1) * 512],
                        ps_qk[:, m * 512:(m + 1) * 512],
                        bqk_sb[:, m:m + 1],
                        None,
                        mybir.AluOpType.add,
                    )
                for mt in range(4):
                    psv = ps_b.tile([P, QW], F32, name="psv", tag="b")
                    for kc in range(KC):
                        nc.tensor.matmul(
                            psv[:, 0:P],
                            lhsT=xtg[:, kc, mt * P:(mt + 1) * P],
                            rhs=wv_sb[:, kc, :],
                            start=(kc == 0),
                            stop=(kc == KC - 1),
                        )
                    ch = rb * 4 + mt
                    nc.vector.tensor_tensor(
                        vext[:, ch, :, 0:64],
                        psv[:, 0:P].rearrange("p (h d) -> p h d", d=64),
                        bv_sb.rearrange("p (h d) -> p h d", d=64),
                        mybir.AluOpType.add,
                    )

            def emit_scores(h, b, qb, c):
                lo = max(0, (c - qb * 8) * P)
                q0 = b * T + qb * QW
                ps_s = ps_a.tile([P, QW], F32, name="ps_s", tag="a")
                for half in range(2):
                    s0 = max(lo, half * 512)
                    s1 = (half + 1) * 512
                    if s0 >= s1:
                        continue
                    nc.tensor.matmul(
                        ps_s[:, s0:s1],
                        lhsT=kT[h * D:(h + 1) * D,
                                b * T + c * P:b * T + (c + 1) * P],
                        rhs=qkT[h * D:(h + 1) * D, q0 + s0:q0 + s1],
                        start=True, stop=True,
                    )
                return ps_s

            def emit_window(h, b, qb):
                nch = 8 * (qb + 1)
                ps_av = ps_b.tile([P, QW], F32, name="ps_av", tag="b")
                ps_s = emit_scores(h, b, qb, 0)
                for c in range(nch):
                    lo = max(0, (c - qb * 8) * P)
                    pt = pt_pool.tile([P, QW], BF16, name="pt")
                    nc.scalar.activation(
                        pt[:, lo:QW], ps_s[:, lo:QW],
                        mybir.ActivationFunctionType.Exp,
                        scale=SCALE,
                    )
                    if c >= qb * 8:
                        nc.vector.tensor_tensor(
                            pt[:, lo:lo + P], pt[:, lo:lo + P],
                            mask_sb[:],
                            mybir.AluOpType.mult,
                        )
                    if c + 1 < nch:
                        ps_s = emit_scores(h, b, qb, c + 1)
                    for half in range(2):
                        s0 = max(lo, half * 512)
                        s1 = (half + 1) * 512
                        if s0 >= s1:
                            continue
                        nc.tensor.matmul(
                            ps_av[:65, s0:s1],
                            lhsT=vext[:, b * 16 + c, h, :],
                            rhs=pt[:, s0:s1],
                            start=(c == 0), stop=(c == nch - 1),
                        )
                stg = stage_pool.tile([65, QW], BF16, name="stg")
                nc.vector.tensor_copy(stg[:], ps_av[:65, :])
                d0 = b * 4 + qb * 2
                nc.sync.dma_start(
                    a2a_in[h].rearrange("(d r) t -> r d t", r=65)[:, d0:d0 + 2, :],
                    stg.rearrange("p (d t) -> p d t", d=2),
                )

            # ---- interleaved phase 1 + head-0 attention ----
            wins = [(0, 0, 0), (0, 0, 1), (0, 1, 0), (0, 1, 1)]
            for i in range(4):
                emit_rb(2 * i)
                emit_rb(2 * i + 1)
                emit_window(*wins[i])
            nc.gpsimd.collective_compute(
                "AllToAll", mybir.AluOpType.bypass,
                ins=[a2a_in[0].opt()], outs=[a2a_out[0].opt()],
                replica_groups=groups,
            )
            # deferred big loads: queue after attention-critical DMAs
            wpr_sb = consts.tile([P, KC, C], BF16)
            nc.sync.dma_start(
                wpr_sb[:], w_pr.ap().rearrange("(kc p) m -> p kc m", p=P)
            )
            bpr_sb = consts.tile([P, C], F32)
            nc.sync.dma_start(
                bpr_sb[:],
                b_pr.ap().rearrange("(o m) -> o m", o=1).to_broadcast((P, C)),
            )
            # ---- head-1 attention + A2A#2 ----
            for b in range(B):
                for qb in range(2):
                    emit_window(1, b, qb)
            nc.gpsimd.collective_compute(
                "AllToAll", mybir.AluOpType.bypass,
                ins=[a2a_in[1].opt()], outs=[a2a_out[1].opt()],
                replica_groups=groups,
            )

            # ---- phase 3: normalize (1/Z) + output projection ----
            att_sb = [
                persist.tile([P, 4, TSH], BF16, name=f"att_sb_{h}")
                for h in range(HPC)
            ]
            rec_rep = [
                persist.tile([P, 4 * TSH], BF16, name=f"rec_rep_{h}")
                for h in range(HPC)
            ]
            rec_dram = [
                dram.tile([8, TSH], BF16, name=f"rec_dram_{h}")
                for h in range(HPC)
            ]
            out_sb = persist.tile([P, 4, C], F32)
            ps_pr = [
                pool.tile([P, C], F32, name=f"ps_pr_{i}", tag=t)
                for i, (pool, t) in enumerate(
                    [(ps_a, "a"), (ps_a, "a"), (ps_b, "b"), (ps_b, "b")]
                )
            ]
            magic_i = consts.tile([32, P], mybir.dt.int32)
            nc.gpsimd.memset(magic_i[:], 0x7EF311C3)
            two32 = consts.tile([32, P], F32)
            nc.gpsimd.memset(two32[:], 2.0)
            for h in range(HPC):
                # the whole normalize pipeline lives on the (otherwise idle)
                # GpSimd queue: its A2A-gated waits cannot head-of-line-block
                # the attention-critical Sync/Vector/Scalar queues
                srcz = a2a_out[h].rearrange(
                    "(kc two df) (qq t) -> two df kc qq t", two=2, df=65, qq=4
                )
                zt = small_pool.tile([32, P], BF16, name="zt")
                zt_eng = nc.gpsimd if h == 0 else nc.scalar
                for half in range(2):
                    zt_eng.dma_start(
                        zt[half * 16:(half + 1) * 16], srcz[half, 64]
                    )
                src = a2a_out[h].rearrange(
                    "(kc two df) t -> two df kc t", two=2, df=65
                )
                att_eng = nc.gpsimd if h == 0 else nc.sync
                for half in range(2):
                    att_eng.dma_start(
                        att_sb[h][half * 64:(half + 1) * 64], src[half, 0:64]
                    )
                rec16 = small_pool.tile([32, P], BF16, name="rec16")
                if h == 0:
                    # 1/Z via bit-trick + 2 Newton steps (Pool has no divide;
                    # DVE mid-h1 must not be blocked by an A2A#1-gated op)
                    ztf = small_pool.tile([32, P], F32, name="ztf")
                    nc.gpsimd.tensor_copy(ztf[:], zt[:])
                    y = small_pool.tile([32, P], F32, name="y")
                    nc.gpsimd.tensor_tensor(
                        y[:].bitcast(mybir.dt.int32), magic_i[:],
                        ztf[:].bitcast(mybir.dt.int32), mybir.AluOpType.subtract,
                    )
                    u = small_pool.tile([32, P], F32, name="u")
                    for _ in range(2):
                        nc.gpsimd.tensor_tensor(
                            u[:], ztf[:], y[:], mybir.AluOpType.mult
                        )
                        nc.gpsimd.tensor_tensor(
                            u[:], two32[:], u[:], mybir.AluOpType.subtract
                        )
                        nc.gpsimd.tensor_tensor(
                            y[:], y[:], u[:], mybir.AluOpType.mult
                        )
                    nc.gpsimd.tensor_copy(rec16[:], y[:])
                    nc.gpsimd.dma_start(
                        rec_dram[h].rearrange("hk (qq t) -> (hk qq) t", qq=4),
                        rec16[:],
                    )
                else:
                    # post-A2A#2 the DVE queue is drained: native reciprocal
                    rec32 = small_pool.tile([32, P], F32, name="rec32")
                    nc.vector.reciprocal(rec32[:], zt[:])
                    nc.vector.tensor_copy(rec16[:], rec32[:])
                    nc.scalar.dma_start(
                        rec_dram[h].rearrange("hk (qq t) -> (hk qq) t", qq=4),
                        rec16[:],
                    )
                rep_eng = nc.gpsimd if h == 0 else nc.scalar
                for half in range(2):
                    rep_eng.dma_start(
                        rec_rep[h][half * 64:(half + 1) * 64],
                        rec_dram[h].rearrange("(half kc) t -> half (kc t)", half=2)
                        [half:half + 1].to_broadcast((64, 4 * TSH)),
                    )
                for kc in range(4):
                    # h0 mults stay on GpSimd (vector must not block mid-h1);
                    # h1 mults go on the by-then-idle DVE (2x faster each)
                    eng = nc.gpsimd if h == 0 else nc.vector
                    eng.tensor_tensor(
                        att_sb[h][:, kc, :],
                        att_sb[h][:, kc, :],
                        rec_rep[h].rearrange("p (kc t) -> p kc t", kc=4)[:, kc, :],
                        mybir.AluOpType.mult,
                    )
                    for mt in range(4):
                        for nb in range(2):
                            nc.tensor.matmul(
                                ps_pr[mt][:, nb * 512:(nb + 1) * 512],
                                lhsT=att_sb[h][:, kc, mt * P:(mt + 1) * P],
                                rhs=wpr_sb[:, h * 4 + kc, nb * 512:(nb + 1) * 512],
                                start=(h == 0 and kc == 0),
                                stop=(h == 1 and kc == 3),
                            )
            for mt in range(4):
                for nb in range(2):
                    nc.vector.tensor_tensor(
                        out_sb[:, mt, nb * 512:(nb + 1) * 512],
                        ps_pr[mt][:, nb * 512:(nb + 1) * 512],
                        bpr_sb[:, nb * 512:(nb + 1) * 512],
                        mybir.AluOpType.add,
                    )
                nc.sync.dma_start(
                    out.ap().rearrange("(mt p) c -> p mt c", p=P)[:, mt],
                    out_sb[:, mt],
                )
    nc.finalize()
    return nc


_NC_CACHE = None


def _get_nc():
    global _NC_CACHE
    if _NC_CACHE is None:
        _NC_CACHE = build_nc()
    return _NC_CACHE


def make_in_maps(x, W_attn, b_attn, W_proj, b_proj):
    bf = ml_dtypes.bfloat16
    x_flat = np.asarray(x, np.float32).reshape(BT, C)
    xT_bf = np.ascontiguousarray(x_flat.T).astype(bf)
    W_attn = np.asarray(W_attn, np.float32)
    b_attn = np.asarray(b_attn, np.float32)
    b_proj = np.asarray(b_proj, np.float32)
    # W_proj rows permuted to A2A delivery order: chunk (h, kc, half) ->
    # head 4*kc + 2*half + h
    perm = np.concatenate(
        [np.arange((4 * kc + 2 * half + h) * D, (4 * kc + 2 * half + h + 1) * D)
         for h in range(2) for kc in range(4) for half in range(2)]
    )
    W_proj_bf = np.ascontiguousarray(np.asarray(W_proj, np.float32)[perm]).astype(bf)
    mask = (np.arange(P)[None, :] >= np.arange(P)[:, None]).astype(bf)

    in_maps = []
    for i in range(NCORES):
        cs = slice(i * P, (i + 1) * P)
        w_qk = np.concatenate(
            [W_attn[:, 0:C][:, cs], W_attn[:, C:2 * C][:, cs]], axis=1
        ).astype(bf)
        b_qk = np.concatenate([b_attn[0:C][cs], b_attn[C:2 * C][cs]])
        in_maps.append({
            "xT": xT_bf,
            "w_qk": np.ascontiguousarray(w_qk),
            "w_v": np.ascontiguousarray(W_attn[:, 2 * C:3 * C][:, cs]).astype(bf),
            "b_qk": np.ascontiguousarray(b_qk.astype(np.float32)),
            "b_v": np.ascontiguousarray(b_attn[2 * C:3 * C][cs]).astype(np.float32),
            "w_proj": W_proj_bf,
            "b_proj": b_proj,
            "mask": mask,
        })
    return in_maps


def kernel(x, W_attn, b_attn, W_proj, b_proj):
    nc = _get_nc()
    in_maps = make_in_maps(x, W_attn, b_attn, W_proj, b_proj)
    res = run_bass_kernel_spmd(nc, in_maps, core_ids=list(range(NCORES)))
    shards = [np.asarray(res.results[i]["out"], np.float32) for i in range(NCORES)]
    return np.concatenate(shards, axis=0).reshape(B, T, C)


# revision 6
# speedup vs baseline: 1.0436x; 1.0436x over previous
"""Distributed causal multi-head attention block on 8 TRN2 NeuronCores.

Tensor-parallel over heads (2 heads/core, core r holds global heads
2r, 2r+1).  Schedule per core:
  - QKV row-block pairs interleaved with head-0 attention windows, so
    ScalarE exp starts ~20us into the run instead of after all of QKV:
    rb0 rb1 W(b0,q0) rb2 rb3 W(b0,q1) rb4 rb5 W(b1,q0) rb6 rb7 W(b1,q1).
  - Attention windows are software-pipelined: scores(c+1) is emitted
    before AV(c) so the PE never stalls behind exp(c) on ScalarE.
    Scores in [k,q] layout, exp on ScalarE (1/8 folded), diagonal
    triangle masked on DVE, AV accumulated over key chunks with a ones
    column in v giving softmax row sums (Z) for free.
  - Raw [65, q] (att|Z) tiles ship UNNORMALIZED through a per-head
    AllToAll; h0's A2A overlaps h1's attention, h1's A2A overlaps h0's
    projection half.  Post-A2A, Z rows are gathered compactly [32, 128],
    reciprocal'd in one DVE op, broadcast via a DRAM-bounce DMA, and
    applied as per-kc bf16 multiplies; the output projection accumulates
    h0's cin chunks then h1's (kc-outer so h1 chunks start as soon as
    their normalize lands), with the t-shard DMA'd out per 128-row block.
"""

import numpy as np
import ml_dtypes

import concourse.bass as bass
import concourse.mybir as mybir
import concourse.tile as tile
from concourse import bacc
from concourse.bass_utils import run_bass_kernel_spmd

P = 128
B, T, C = 2, 2048, 1024
H, D = 16, 64
NCORES = 8
HPC = 2                    # heads per core
BT = B * T                 # 4096
KC = C // P                # 8 contraction chunks
NRB = BT // 512            # 8 row blocks of 512
QW = 1024                  # query window
TSH = 512                  # output t-shard rows per core
F32 = mybir.dt.float32
BF16 = mybir.dt.bfloat16
SCALE = 1.0 / 8.0


def build_nc():
    nc = bacc.Bacc(None, target_bir_lowering=False)

    xT = nc.dram_tensor("xT", [C, BT], BF16, kind="ExternalInput")
    w_qk = nc.dram_tensor("w_qk", [C, 2 * P], BF16, kind="ExternalInput")
    w_v = nc.dram_tensor("w_v", [C, P], BF16, kind="ExternalInput")
    b_qk = nc.dram_tensor("b_qk", [2 * P], F32, kind="ExternalInput")
    b_v = nc.dram_tensor("b_v", [P], F32, kind="ExternalInput")
    w_pr = nc.dram_tensor("w_proj", [C, C], BF16, kind="ExternalInput")
    b_pr = nc.dram_tensor("b_proj", [C], F32, kind="ExternalInput")
    maskm = nc.dram_tensor("mask", [P, P], BF16, kind="ExternalInput")
    out = nc.dram_tensor("out", [TSH, C], F32, kind="ExternalOutput")

    groups = [list(range(NCORES))]

    with tile.TileContext(nc) as tc:
        with (
            tc.tile_pool(name="consts", bufs=1) as consts,
            tc.tile_pool(name="persist", bufs=1) as persist,
            tc.tile_pool(name="xtg", bufs=3) as xtg_pool,
            tc.tile_pool(name="pt", bufs=3) as pt_pool,
            tc.tile_pool(name="stage", bufs=3) as stage_pool,
            tc.tile_pool(name="small", bufs=2) as small_pool,
            tc.tile_pool(name="ps_a", bufs=2, space="PSUM") as ps_a,
            tc.tile_pool(name="ps_b", bufs=2, space="PSUM") as ps_b,
            tc.tile_pool(name="dram", bufs=1, space="DRAM") as dram,
        ):
            # ---- constants (W_proj + b_proj deferred until after A2A#1) ----
            wqk_sb = consts.tile([P, KC, 2 * P], BF16)
            nc.sync.dma_start(wqk_sb[:], w_qk.ap().rearrange("(kc p) m -> p kc m", p=P))
            wv_sb = consts.tile([P, KC, P], BF16)
            nc.sync.dma_start(wv_sb[:], w_v.ap().rearrange("(kc p) m -> p kc m", p=P))
            bqk_sb = consts.tile([P, 2], F32)
            nc.sync.dma_start(bqk_sb[:], b_qk.ap().rearrange("(m p) -> p m", p=P))
            bv_sb = consts.tile([P, P], F32)
            nc.sync.dma_start(
                bv_sb[:],
                b_v.ap().rearrange("(o m) -> o m", o=1).to_broadcast((P, P)),
            )
            mask_sb = consts.tile([P, P], BF16)
            nc.sync.dma_start(mask_sb[:], maskm.ap())

            # persistent phase-1 outputs
            # qkT/kT: [p=(h,d), t];  vext: [p=t%128, chunk, head, 65]
            qkT = persist.tile([P, BT], BF16)
            kT = persist.tile([P, BT], BF16)
            vext = persist.tile([P, BT // P, HPC, 65], BF16)
            nc.vector.memset(vext[:, :, :, 64], 1.0)

            xT_blocked = xT.ap().rearrange(
                "(kh kc p) (r t) -> r kh p kc t", p=P, r=NRB, kh=2
            )

            a2a_in = [
                dram.tile([NCORES * 65, TSH], BF16, name=f"a2a_in_{h}")
                for h in range(HPC)
            ]
            a2a_out = [
                dram.tile([NCORES * 65, TSH], BF16, name=f"a2a_out_{h}")
                for h in range(HPC)
            ]

            def emit_rb(rb):
                xtg = xtg_pool.tile([P, KC, 512], BF16, name="xtg")
                nc.sync.dma_start(xtg[:, 0:KC // 2, :], xT_blocked[rb, 0])
                nc.sync.dma_start(xtg[:, KC // 2:KC, :], xT_blocked[rb, 1])
                ps_qk = ps_a.tile([P, QW], F32, name="ps_qk", tag="a")
                for kc in range(KC):
                    for m in range(2):
                        nc.tensor.matmul(
                            ps_qk[:, m * 512:(m + 1) * 512],
                            lhsT=wqk_sb[:, kc, m * P:(m + 1) * P],
                            rhs=xtg[:, kc, :],
                            start=(kc == 0),
                            stop=(kc == KC - 1),
                        )
                for m, dst in ((0, qkT), (1, kT)):
                    nc.vector.tensor_scalar(
                        dst[:, rb * 512:(rb + 1) * 512],
                        ps_qk[:, m * 512:(m + 1) * 512],
                        bqk_sb[:, m:m + 1],
                        None,
                        mybir.AluOpType.add,
                    )
                for mt in range(4):
                    psv = ps_b.tile([P, QW], F32, name="psv", tag="b")
                    for kc in range(KC):
                        nc.tensor.matmul(
                            psv[:, 0:P],
                            lhsT=xtg[:, kc, mt * P:(mt + 1) * P],
                            rhs=wv_sb[:, kc, :],
                            start=(kc == 0),
                            stop=(kc == KC - 1),
                        )
                    ch = rb * 4 + mt
                    nc.vector.tensor_tensor(
                        vext[:, ch, :, 0:64],
                        psv[:, 0:P].rearrange("p (h d) -> p h d", d=64),
                        bv_sb.rearrange("p (h d) -> p h d", d=64),
                        mybir.AluOpType.add,
                    )

            def emit_scores(h, b, qb, c):
                lo = max(0, (c - qb * 8) * P)
                q0 = b * T + qb * QW
                ps_s = ps_a.tile([P, QW], F32, name="ps_s", tag="a")
                for half in range(2):
                    s0 = max(lo, half * 512)
                    s1 = (half + 1) * 512
                    if s0 >= s1:
                        continue
                    nc.tensor.matmul(
                        ps_s[:, s0:s1],
                        lhsT=kT[h * D:(h + 1) * D,
                                b * T + c * P:b * T + (c + 1) * P],
                        rhs=qkT[h * D:(h + 1) * D, q0 + s0:q0 + s1],
                        start=True, stop=True,
                    )
                return ps_s

            def emit_window(h, b, qb):
                nch = 8 * (qb + 1)
                ps_av = ps_b.tile([P, QW], F32, name="ps_av", tag="b")
                ps_s = emit_scores(h, b, qb, 0)
                for c in range(nch):
                    lo = max(0, (c - qb * 8) * P)
                    pt = pt_pool.tile([P, QW], BF16, name="pt")
                    nc.scalar.activation(
                        pt[:, lo:QW], ps_s[:, lo:QW],
                        mybir.ActivationFunctionType.Exp,
                        scale=SCALE,
                    )
                    if c >= qb * 8:
                        nc.vector.tensor_tensor(
                            pt[:, lo:lo + P], pt[:, lo:lo + P],
                            mask_sb[:],
                            mybir.AluOpType.mult,
                        )
                    if c + 1 < nch:
                        ps_s = emit_scores(h, b, qb, c + 1)
                    for half in range(2):
                        s0 = max(lo, half * 512)
                        s1 = (half + 1) * 512
                        if s0 >= s1:
                            continue
                        nc.tensor.matmul(
                            ps_av[:65, s0:s1],
                            lhsT=vext[:, b * 16 + c, h, :],
                            rhs=pt[:, s0:s1],
                            start=(c == 0), stop=(c == nch - 1),
                        )
                stg = stage_pool.tile([65, QW], BF16, name="stg")
                nc.vector.tensor_copy(stg[:], ps_av[:65, :])
                d0 = b * 4 + qb * 2
                nc.sync.dma_start(
                    a2a_in[h].rearrange("(d r) t -> r d t", r=65)[:, d0:d0 + 2, :],
                    stg.rearrange("p (d t) -> p d t", d=2),
                )

            # ---- interleaved phase 1 + head-0 attention ----
            wins = [(0, 0, 0), (0, 0, 1), (0, 1, 0), (0, 1, 1)]
            for i in range(4):
                emit_rb(2 * i)
                emit_rb(2 * i + 1)
                emit_window(*wins[i])
            nc.gpsimd.collective_compute(
                "AllToAll", mybir.AluOpType.bypass,
                ins=[a2a_in[0].opt()], outs=[a2a_out[0].opt()],
                replica_groups=groups,
            )
            # deferred big loads: queue after attention-critical DMAs
            wpr_sb = consts.tile([P, KC, C], BF16)
            nc.sync.dma_start(
                wpr_sb[:], w_pr.ap().rearrange("(kc p) m -> p kc m", p=P)
            )
            bpr_sb = consts.tile([P, C], F32)
            nc.sync.dma_start(
                bpr_sb[:],
                b_pr.ap().rearrange("(o m) -> o m", o=1).to_broadcast((P, C)),
            )
            # ---- head-1 attention + A2A#2 ----
            for b in range(B):
                for qb in range(2):
                    emit_window(1, b, qb)
            nc.gpsimd.collective_compute(
                "AllToAll", mybir.AluOpType.bypass,
                ins=[a2a_in[1].opt()], outs=[a2a_out[1].opt()],
                replica_groups=groups,
            )

            # ---- phase 3: normalize (1/Z) + output projection ----
            att_sb = [
                persist.tile([P, 4, TSH], BF16, name=f"att_sb_{h}")
                for h in range(HPC)
            ]
            rec_rep = [
                persist.tile([P, 4 * TSH], BF16, name=f"rec_rep_{h}")
                for h in range(HPC)
            ]
            rec_dram = [
                dram.tile([8, TSH], BF16, name=f"rec_dram_{h}")
                for h in range(HPC)
            ]
            out_sb = persist.tile([P, 4, C], F32)
            ps_pr = [
                pool.tile([P, C], F32, name=f"ps_pr_{i}", tag=t)
                for i, (pool, t) in enumerate(
                    [(ps_a, "a"), (ps_a, "a"), (ps_b, "b"), (ps_b, "b")]
                )
            ]
            magic_i = consts.tile([32, P], mybir.dt.int32)
            nc.gpsimd.memset(magic_i[:], 0x7EF311C3)
            two32 = consts.tile([32, P], F32)
            nc.gpsimd.memset(two32[:], 2.0)
            for h in range(HPC):
                # the whole normalize pipeline lives on the (otherwise idle)
                # GpSimd queue: its A2A-gated waits cannot head-of-line-block
                # the attention-critical Sync/Vector/Scalar queues
                srcz = a2a_out[h].rearrange(
                    "(kc two df) (qq t) -> two df kc qq t", two=2, df=65, qq=4
                )
                zt = small_pool.tile([32, P], BF16, name="zt")
                zt_eng = nc.gpsimd if h == 0 else nc.scalar
                for half in range(2):
                    zt_eng.dma_start(
                        zt[half * 16:(half + 1) * 16], srcz[half, 64]
                    )
                src = a2a_out[h].rearrange(
                    "(kc two df) t -> two df kc t", two=2, df=65
                )
                att_eng = nc.gpsimd if h == 0 else nc.sync
                for half in range(2):
                    att_eng.dma_start(
                        att_sb[h][half * 64:(half + 1) * 64], src[half, 0:64]
                    )
                rec16 = small_pool.tile([32, P], BF16, name="rec16")
                if h == 0:
                    # 1/Z via bit-trick + 2 Newton steps (Pool has no divide;
                    # DVE mid-h1 must not be blocked by an A2A#1-gated op)
                    ztf = small_pool.tile([32, P], F32, name="ztf")
                    nc.gpsimd.tensor_copy(ztf[:], zt[:])
                    y = small_pool.tile([32, P], F32, name="y")
                    nc.gpsimd.tensor_tensor(
                        y[:].bitcast(mybir.dt.int32), magic_i[:],
                        ztf[:].bitcast(mybir.dt.int32), mybir.AluOpType.subtract,
                    )
                    u = small_pool.tile([32, P], F32, name="u")
                    for _ in range(2):
                        nc.gpsimd.tensor_tensor(
                            u[:], ztf[:], y[:], mybir.AluOpType.mult
                        )
                        nc.gpsimd.tensor_tensor(
                            u[:], two32[:], u[:], mybir.AluOpType.subtract
                        )
                        nc.gpsimd.tensor_tensor(
                            y[:], y[:], u[:], mybir.AluOpType.mult
                        )
                    nc.gpsimd.tensor_copy(rec16[:], y[:])
                    nc.gpsimd.dma_start(
                        rec_dram[h].rearrange("hk (qq t) -> (hk qq) t", qq=4),
                        rec16[:],
                    )
                else:
                    # post-A2A#2 the DVE queue is drained: native reciprocal
                    rec32 = small_pool.tile([32, P], F32, name="rec32")
                    nc.vector.reciprocal(rec32[:], zt[:])
                    nc.vector.tensor_copy(rec16[:], rec32[:])
                    nc.scalar.dma_start(
                        rec_dram[h].rearrange("hk (qq t) -> (hk qq) t", qq=4),
                        rec16[:],
                    )
                if h == 0:
                    for half in range(2):
                        nc.gpsimd.dma_start(
                            rec_rep[h][half * 64:(half + 1) * 64],
                            rec_dram[h].rearrange(
                                "(half kc) t -> half (kc t)", half=2
                            )[half:half + 1].to_broadcast((64, 4 * TSH)),
                        )
                else:
                    # per-kc slices on alternating queues: mult(kc) and the
                    # first proj chunk gate on 64KB, not the full 512KB
                    rep_kc = rec_rep[h].rearrange("p (kc t) -> p kc t", kc=4)
                    srcr = rec_dram[h].rearrange(
                        "(half kc) t -> half kc t", half=2
                    )
                    for kc in range(4):
                        for half in range(2):
                            eng = nc.scalar if half == 0 else nc.sync
                            eng.dma_start(
                                rep_kc[half * 64:(half + 1) * 64, kc, :],
                                srcr[half:half + 1, kc, :].to_broadcast(
                                    (64, TSH)
                                ),
                            )
                for kc in range(4):
                    # h0 mults stay on GpSimd (vector must not block mid-h1);
                    # h1 mults go on the by-then-idle DVE (2x faster each)
                    eng = nc.gpsimd if h == 0 else nc.vector
                    eng.tensor_tensor(
                        att_sb[h][:, kc, :],
                        att_sb[h][:, kc, :],
                        rec_rep[h].rearrange("p (kc t) -> p kc t", kc=4)[:, kc, :],
                        mybir.AluOpType.mult,
                    )
                    for mt in range(4):
                        for nb in range(2):
                            nc.tensor.matmul(
                                ps_pr[mt][:, nb * 512:(nb + 1) * 512],
                                lhsT=att_sb[h][:, kc, mt * P:(mt + 1) * P],
                                rhs=wpr_sb[:, h * 4 + kc, nb * 512:(nb + 1) * 512],
                                start=(h == 0 and kc == 0),
                                stop=(h == 1 and kc == 3),
                            )
            for mt in range(4):
                for nb in range(2):
                    nc.vector.tensor_tensor(
                        out_sb[:, mt, nb * 512:(nb + 1) * 512],
                        ps_pr[mt][:, nb * 512:(nb + 1) * 512],
                        bpr_sb[:, nb * 512:(nb + 1) * 512],
                        mybir.AluOpType.add,
                    )
                nc.sync.dma_start(
                    out.ap().rearrange("(mt p) c -> p mt c", p=P)[:, mt],
                    out_sb[:, mt],
                )
    nc.finalize()
    return nc


_NC_CACHE = None


def _get_nc():
    global _NC_CACHE
    if _NC_CACHE is None:
        _NC_CACHE = build_nc()
    return _NC_CACHE


def make_in_maps(x, W_attn, b_attn, W_proj, b_proj):
    bf = ml_dtypes.bfloat16
    x_flat = np.asarray(x, np.float32).reshape(BT, C)
    xT_bf = np.ascontiguousarray(x_flat.T).astype(bf)
    W_attn = np.asarray(W_attn, np.float32)
    b_attn = np.asarray(b_attn, np.float32)
    b_proj = np.asarray(b_proj, np.float32)
    # W_proj rows permuted to A2A delivery order: chunk (h, kc, half) ->
    # head 4*kc + 2*half + h
    perm = np.concatenate(
        [np.arange((4 * kc + 2 * half + h) * D, (4 * kc + 2 * half + h + 1) * D)
         for h in range(2) for kc in range(4) for half in range(2)]
    )
    W_proj_bf = np.ascontiguousarray(np.asarray(W_proj, np.float32)[perm]).astype(bf)
    mask = (np.arange(P)[None, :] >= np.arange(P)[:, None]).astype(bf)

    in_maps = []
    for i in range(NCORES):
        cs = slice(i * P, (i + 1) * P)
        w_qk = np.concatenate(
            [W_attn[:, 0:C][:, cs], W_attn[:, C:2 * C][:, cs]], axis=1
        ).astype(bf)
        b_qk = np.concatenate([b_attn[0:C][cs], b_attn[C:2 * C][cs]])
        in_maps.append({
            "xT": xT_bf,
            "w_qk": np.ascontiguousarray(w_qk),
            "w_v": np.ascontiguousarray(W_attn[:, 2 * C:3 * C][:, cs]).astype(bf),
            "b_qk": np.ascontiguousarray(b_qk.astype(np.float32)),
            "b_v": np.ascontiguousarray(b_attn[2 * C:3 * C][cs]).astype(np.float32),
            "w_proj": W_proj_bf,
            "b_proj": b_proj,
            "mask": mask,
        })
    return in_maps


def kernel(x, W_attn, b_attn, W_proj, b_proj):
    nc = _get_nc()
    in_maps = make_in_maps(x, W_attn, b_attn, W_proj, b_proj)
    res = run_bass_kernel_spmd(nc, in_maps, core_ids=list(range(NCORES)))
    shards = [np.asarray(res.results[i]["out"], np.float32) for i in range(NCORES)]
    return np.concatenate(shards, axis=0).reshape(B, T, C)
